# revision 25
# baseline (speedup 1.0000x reference)
"""CAM (channel-attention) kernel for Trainium2, data-parallel over batch on 8 cores.

Reference computation (per sample b):
    avg[c] = mean over spatial of x[b, c, :, :]
    mx[c]  = max  over spatial of x[b, c, :, :]
    gate   = sigmoid(W2 @ relu(W1 @ avg) + W2 @ relu(W1 @ mx))
    y[b]   = x[b] * gate[:, None, None]

Design (per core, 4 samples; memory-roofline bound at ~360 GB/s/core):
  - Each sample is SBUF-resident as [128 partitions, 4 channel-groups, 3136]
    (channel c = ci*128 + p): x is read from HBM once and written back once,
    the minimum possible traffic (~51 MB/core -> ~140 us at roofline).
  - Input DMAs ride the SP HW-DGE ring, output DMAs the Pool SWDGE ring:
    a DMA occupies its issuing ring for the whole transfer (~4.8 us per
    1.6 MB), so input and output streams must live on different rings, and
    the ACT ring is kept free for the mean-reduction activations.
  - The per-channel mean comes free on ScalarE: activation(Copy, scale=1/S)
    with accum_out, while VectorE does reduce_max. The scratch `out` of that
    activation is a small rotating dummy tile.
  - The tiny shared MLP runs on TensorE with host-pretransposed weights.
    relu(W1@mean) = relu(W1@sum)/S (positive homogeneity) and W2 is linear,
    so both branches merge into one [32,1] vector before layer 2:
        layer1: psum[32,2] += w1t[:,ci,:].T @ [mean | max], ci=0..3
        hsum = relu(psum)[:,0] + relu(psum)[:,1]
        layer2 per ci: p2[:,ci] = w2t[:,ci*128:].T @ hsum ; gate = sigmoid(p2)
  - Gating multiply on VectorE tensor_scalar (2x fp32 SBUF mode) into fresh
    output tiles, then streamed out per channel-group.
  - A zeroing matmul opens each PSUM accumulation group and warm-up matmuls
    touch the weight tiles once: every hot instruction then needs at most
    one semaphore wait, minimizing Bacc's EventSemaphore splitting (this
    toolchain allows exactly ONE wait slot per hardware instruction).
  - Built on Bacc: nc.compile() legalizes any remaining multi-wait
    instructions into EventSemaphore prefixes.
"""

import numpy as np

import concourse.bacc as bacc
import concourse.bass as bass
import concourse.tile as tile
from concourse import mybir

N_CORES = 8
B = 32
C = 512
S = 56 * 56  # 3136
BPC = B // N_CORES  # samples per core
P = 128
CI = C // P  # channel groups of 128
HID = 32

F32 = mybir.dt.float32
AF = mybir.ActivationFunctionType

LAST_RESULTS = None  # BassKernelResults of the most recent run (for test harness)
_NC_CACHE = None


def _build_bass():
    nc = bacc.Bacc()
    x = nc.dram_tensor("x", (BPC, CI, P, S), F32, kind="ExternalInput")
    w1t = nc.dram_tensor("w1t", (P, CI, HID), F32, kind="ExternalInput")
    w2t = nc.dram_tensor("w2t", (HID, C), F32, kind="ExternalInput")
    y = nc.dram_tensor("y", (BPC, CI, P, S), F32, kind="ExternalOutput")

    with tile.TileContext(nc) as tc:
        with (
            tc.tile_pool(name="xp", bufs=2) as xp,
            tc.tile_pool(name="yp", bufs=4) as yp,
            tc.tile_pool(name="consts", bufs=1) as consts,
            tc.tile_pool(name="small", bufs=4) as small,
            tc.tile_pool(name="dump", bufs=2) as dump,
            tc.tile_pool(name="ps1", bufs=4, space=bass.MemorySpace.PSUM) as ps1,
            tc.tile_pool(name="ps2", bufs=4, space=bass.MemorySpace.PSUM) as ps2,
        ):
            w1t_sb = consts.tile([P, CI, HID], F32)
            nc.sync.dma_start(out=w1t_sb[:], in_=w1t[:])
            w2t_sb = consts.tile([HID, C], F32)
            nc.sync.dma_start(out=w2t_sb[:], in_=w2t[:])
            zeros = consts.tile([P, CI], F32)
            nc.vector.memset(zeros[:], 0.0)

            # PE observes the two weight-DMA semaphores here, once.
            pw = ps1.tile([HID, 2], F32, tag="p1", name="pw")
            nc.tensor.matmul(pw[:, 0:1], w1t_sb[:, 0, :], w1t_sb[:, 0, 0:1])
            pw2 = ps2.tile([P, CI], F32, tag="p2", name="pw2")
            nc.tensor.matmul(pw2[:, 0:1], w2t_sb[:, 0:P], w2t_sb[:, 0:1])

            for b in range(BPC):
                xt = xp.tile([P, CI, S], F32, tag="xt", name=f"xt{b}")
                stats = small.tile([P, CI, 2], F32, tag="stats", name=f"st{b}")
                for ci in range(CI):
                    nc.sync.dma_start(out=xt[:, ci, :], in_=x[b, ci])
                for ci in range(CI):
                    # ScalarE: accum_out = sum(x/S) = mean; dmy is scratch
                    dmy = dump.tile([P, S], F32, tag="dmy", name=f"dmy{b}_{ci}")
                    nc.scalar.activation(
                        out=dmy[:],
                        in_=xt[:, ci, :],
                        func=AF.Copy,
                        scale=1.0 / S,
                        accum_out=stats[:, ci, 0:1],
                    )
                    nc.vector.reduce_max(
                        out=stats[:, ci, 1:2],
                        in_=xt[:, ci, :],
                        axis=mybir.AxisListType.X,
                    )

                # layer 1: psum [32, 2] = sum_ci W1[:, ci-block] @ [mean | max]
                p1 = ps1.tile([HID, 2], F32, tag="p1", name=f"p1_{b}")
                nc.tensor.matmul(
                    p1[:], w1t_sb[:, 0, :], zeros[:, 0:2], start=True, stop=False
                )
                for ci in range(CI):
                    nc.tensor.matmul(
                        p1[:],
                        w1t_sb[:, ci, :],
                        stats[:, ci, :],
                        start=False,
                        stop=(ci == CI - 1),
                    )
                h = small.tile([HID, 2], F32, tag="h", name=f"h{b}")
                nc.scalar.activation(out=h[:], in_=p1[:], func=AF.Relu)
                # W2 is linear: merge branches before layer 2
                hsum = small.tile([HID, 1], F32, tag="hsum", name=f"hs{b}")
                nc.vector.tensor_scalar(
                    out=hsum[:],
                    in0=h[:, 0:1],
                    scalar1=1.0,
                    scalar2=h[:, 1:2],
                    op0=mybir.AluOpType.mult,
                    op1=mybir.AluOpType.add,
                )

                # layer 2: four matmuls into disjoint columns of one PSUM bank
                p2 = ps2.tile([P, CI], F32, tag="p2", name=f"p2_{b}")
                nc.tensor.matmul(
                    p2[:],
                    w2t_sb[:, 0:P],
                    zeros[:HID, 0:CI],
                    start=True,
                    stop=False,
                    skip_group_check=True,
                )
                for ci in range(CI):
                    nc.tensor.matmul(
                        p2[:, ci : ci + 1],
                        w2t_sb[:, ci * P : (ci + 1) * P],
                        hsum[:],
                        start=False,
                        stop=(ci == CI - 1),
                        skip_group_check=True,
                    )
                g = small.tile([P, CI], F32, tag="g", name=f"g{b}")
                nc.scalar.activation(out=g[:], in_=p2[:], func=AF.Sigmoid)
                # single-producer copy: the muls wait on DVE only
                g2 = small.tile([P, CI], F32, tag="g2", name=f"g2_{b}")
                nc.vector.tensor_copy(out=g2[:], in_=g[:])

                for ci in range(CI):
                    yt = yp.tile([P, S], F32, tag="yt", name=f"yt{b}_{ci}")
                    nc.vector.tensor_scalar_mul(
                        out=yt[:],
                        in0=xt[:, ci, :],
                        scalar1=g2[:, ci : ci + 1],
                    )
                    nc.gpsimd.dma_start(out=y[b, ci], in_=yt[:])
    nc.compile()
    return nc


_RUNNER = None


def _make_runner(nc):
    """jit(shard_map) over the bass_exec custom call — the same lowering
    run_bass_kernel_spmd uses under axon, but built once and cached so
    repeated kernel() calls reuse one loaded executable (loading a second
    copy of the NEFF in the same process wedges the device)."""
    import jax
    from jax.sharding import Mesh, PartitionSpec
    from jax.experimental.shard_map import shard_map
    from concourse.bass2jax import (
        _bass_exec_p,
        install_neuronx_cc_hook,
        partition_id_tensor,
    )

    install_neuronx_cc_hook()
    partition_name = nc.partition_id_tensor.name if nc.partition_id_tensor else None
    in_names, out_names, out_avals = [], [], []
    for alloc in nc.m.functions[0].allocations:
        if not isinstance(alloc, mybir.MemoryLocationSet):
            continue
        name = alloc.memorylocations[0].name
        if alloc.kind == "ExternalInput":
            if name != partition_name:
                in_names.append(name)
        elif alloc.kind == "ExternalOutput":
            out_names.append(name)
            out_avals.append(
                jax.core.ShapedArray(
                    tuple(alloc.tensor_shape), mybir.dt.np(alloc.dtype)
                )
            )
    all_in = in_names + out_names
    if partition_name is not None:
        all_in.append(partition_name)

    def _body(*args):
        operands = list(args)
        if partition_name is not None:
            operands.append(partition_id_tensor())
        outs = _bass_exec_p.bind(
            *operands,
            out_avals=tuple(out_avals),
            in_names=tuple(all_in),
            out_names=tuple(out_names),
            lowering_input_output_aliases=(),
            sim_require_finite=True,
            sim_require_nnan=True,
            nc=nc,
        )
        return tuple(outs)

    devices = jax.devices()[:N_CORES]
    mesh = Mesh(np.asarray(devices), ("core",))
    n_args = len(in_names) + len(out_names)
    fn = jax.jit(
        shard_map(
            _body,
            mesh=mesh,
            in_specs=(PartitionSpec("core"),) * n_args,
            out_specs=(PartitionSpec("core"),) * len(out_names),
            check_rep=False,
        ),
        keep_unused=True,
    )
    assert in_names == ["x", "w1t", "w2t"] and out_names == ["y"], (
        in_names,
        out_names,
    )
    return fn


def kernel(x, w1, w2, **_ignored):
    global _NC_CACHE, _RUNNER
    x = np.ascontiguousarray(np.asarray(x, dtype=np.float32))
    w1 = np.asarray(w1, dtype=np.float32)  # [HID, C]
    w2 = np.asarray(w2, dtype=np.float32)  # [C, HID]

    # SBUF layouts, pretransposed on host
    w1t = np.ascontiguousarray(
        w1.T.reshape(CI, P, HID).transpose(1, 0, 2)
    )  # [P, CI, HID]; w1t[p, ci, h] = w1[h, ci*128+p]
    w2t = np.ascontiguousarray(w2.T)  # [HID, C]

    if _NC_CACHE is None:
        _NC_CACHE = _build_bass()
    if _RUNNER is None:
        _RUNNER = _make_runner(_NC_CACHE)

    # global inputs concatenated on axis 0; shard_map hands each core its slice
    xs = x.reshape(N_CORES * BPC, CI, P, S)
    w1ts = np.concatenate([w1t] * N_CORES, axis=0)
    w2ts = np.concatenate([w2t] * N_CORES, axis=0)
    ybuf = np.zeros_like(xs)
    (y,) = _RUNNER(xs, w1ts, w2ts, ybuf)
    return np.asarray(y).reshape(B, C, 56, 56)


# revision 28
# speedup vs baseline: 1.1411x; 1.1411x over previous
"""CAM (channel-attention) kernel for Trainium2, data-parallel over batch on 8 cores.

Reference computation (per sample b):
    avg[c] = mean over spatial of x[b, c, :, :]
    mx[c]  = max  over spatial of x[b, c, :, :]
    gate   = sigmoid(W2 @ relu(W1 @ avg) + W2 @ relu(W1 @ mx))
    y[b]   = x[b] * gate[:, None, None]

Design (per core, 4 samples; memory-roofline bound at ~360 GB/s/core):
  - Each sample is SBUF-resident as [128 partitions, 4 channel-groups, 3136]
    (channel c = ci*128 + p): x is read from HBM once and written back once,
    the minimum possible traffic (~51 MB/core -> ~140 us at roofline).
  - Input DMAs ride the SP HW-DGE ring, output DMAs the Pool SWDGE ring:
    a DMA occupies its issuing ring for the whole transfer (~4.8 us per
    1.6 MB), so input and output streams must live on different rings, and
    the ACT ring is kept free for the mean-reduction activations. At the
    edges (first sample's loads, last sample's stores) the otherwise-idle
    ACT ring takes half the transfers, halving pipeline fill and drain.
  - The per-channel mean comes free on ScalarE: activation(Copy, scale=1/S)
    with accum_out, while VectorE does reduce_max. The scratch `out` of that
    activation is a small rotating dummy tile.
  - The tiny shared MLP runs on TensorE with host-pretransposed weights.
    relu(W1@mean) = relu(W1@sum)/S (positive homogeneity) and W2 is linear,
    so both branches merge into one [32,1] vector before layer 2:
        layer1: psum[32,2] += w1t[:,ci,:].T @ [mean | max], ci=0..3
        hsum = relu(psum)[:,0] + relu(psum)[:,1]
        layer2 per ci: p2[:,ci] = w2t[:,ci*128:].T @ hsum ; gate = sigmoid(p2)
  - Gating multiply on VectorE tensor_scalar (2x fp32 SBUF mode) into fresh
    output tiles, then streamed out per channel-group.
  - A zeroing matmul opens each PSUM accumulation group and warm-up matmuls
    touch the weight tiles once: every hot instruction then needs at most
    one semaphore wait, minimizing Bacc's EventSemaphore splitting (this
    toolchain allows exactly ONE wait slot per hardware instruction).
  - Built on Bacc: nc.compile() legalizes any remaining multi-wait
    instructions into EventSemaphore prefixes.
"""

import numpy as np

import concourse.bacc as bacc
import concourse.bass as bass
import concourse.tile as tile
from concourse import mybir

N_CORES = 8
B = 32
C = 512
S = 56 * 56  # 3136
BPC = B // N_CORES  # samples per core
P = 128
CI = C // P  # channel groups of 128
HID = 32

F32 = mybir.dt.float32
AF = mybir.ActivationFunctionType

LAST_RESULTS = None  # BassKernelResults of the most recent run (for test harness)
_NC_CACHE = None


def _build_bass():
    nc = bacc.Bacc()
    x = nc.dram_tensor("x", (BPC, CI, P, S), F32, kind="ExternalInput")
    w1t = nc.dram_tensor("w1t", (P, CI, HID), F32, kind="ExternalInput")
    w2t = nc.dram_tensor("w2t", (HID, C), F32, kind="ExternalInput")
    y = nc.dram_tensor("y", (BPC, CI, P, S), F32, kind="ExternalOutput")

    with tile.TileContext(nc) as tc:
        with (
            tc.tile_pool(name="xp", bufs=2) as xp,
            tc.tile_pool(name="yp", bufs=4) as yp,
            tc.tile_pool(name="consts", bufs=1) as consts,
            tc.tile_pool(name="small", bufs=4) as small,
            tc.tile_pool(name="dump", bufs=2) as dump,
            tc.tile_pool(name="ps1", bufs=4, space=bass.MemorySpace.PSUM) as ps1,
            tc.tile_pool(name="ps2", bufs=4, space=bass.MemorySpace.PSUM) as ps2,
        ):
            w1t_sb = consts.tile([P, CI, HID], F32)
            nc.sync.dma_start(out=w1t_sb[:], in_=w1t[:])
            w2t_sb = consts.tile([HID, C], F32)
            nc.sync.dma_start(out=w2t_sb[:], in_=w2t[:])
            zeros = consts.tile([P, CI], F32)
            nc.vector.memset(zeros[:], 0.0)

            # PE observes the two weight-DMA semaphores here, once.
            pw = ps1.tile([HID, 2], F32, tag="p1", name="pw")
            nc.tensor.matmul(pw[:, 0:1], w1t_sb[:, 0, :], w1t_sb[:, 0, 0:1])
            pw2 = ps2.tile([P, CI], F32, tag="p2", name="pw2")
            nc.tensor.matmul(pw2[:, 0:1], w2t_sb[:, 0:P], w2t_sb[:, 0:1])

            for b in range(BPC):
                xt = xp.tile([P, CI, S], F32, tag="xt", name=f"xt{b}")
                stats = small.tile([P, CI, 2], F32, tag="stats", name=f"st{b}")
                for ci in range(CI):
                    in_eng = nc.scalar if (b == 0 and ci % 2 == 1) else nc.sync
                    in_eng.dma_start(out=xt[:, ci, :], in_=x[b, ci])
                for ci in range(CI):
                    # ScalarE: accum_out = sum(x/S) = mean; dmy is scratch
                    dmy = dump.tile([P, S], F32, tag="dmy", name=f"dmy{b}_{ci}")
                    nc.scalar.activation(
                        out=dmy[:],
                        in_=xt[:, ci, :],
                        func=AF.Copy,
                        scale=1.0 / S,
                        accum_out=stats[:, ci, 0:1],
                    )
                    nc.vector.reduce_max(
                        out=stats[:, ci, 1:2],
                        in_=xt[:, ci, :],
                        axis=mybir.AxisListType.X,
                    )

                # layer 1: psum [32, 2] = sum_ci W1[:, ci-block] @ [mean | max]
                p1 = ps1.tile([HID, 2], F32, tag="p1", name=f"p1_{b}")
                nc.tensor.matmul(
                    p1[:], w1t_sb[:, 0, :], zeros[:, 0:2], start=True, stop=False
                )
                for ci in range(CI):
                    nc.tensor.matmul(
                        p1[:],
                        w1t_sb[:, ci, :],
                        stats[:, ci, :],
                        start=False,
                        stop=(ci == CI - 1),
                    )
                h = small.tile([HID, 2], F32, tag="h", name=f"h{b}")
                nc.scalar.activation(out=h[:], in_=p1[:], func=AF.Relu)
                # W2 is linear: merge branches before layer 2
                hsum = small.tile([HID, 1], F32, tag="hsum", name=f"hs{b}")
                nc.vector.tensor_scalar(
                    out=hsum[:],
                    in0=h[:, 0:1],
                    scalar1=1.0,
                    scalar2=h[:, 1:2],
                    op0=mybir.AluOpType.mult,
                    op1=mybir.AluOpType.add,
                )

                # layer 2: four matmuls into disjoint columns of one PSUM bank
                p2 = ps2.tile([P, CI], F32, tag="p2", name=f"p2_{b}")
                nc.tensor.matmul(
                    p2[:],
                    w2t_sb[:, 0:P],
                    zeros[:HID, 0:CI],
                    start=True,
                    stop=False,
                    skip_group_check=True,
                )
                for ci in range(CI):
                    nc.tensor.matmul(
                        p2[:, ci : ci + 1],
                        w2t_sb[:, ci * P : (ci + 1) * P],
                        hsum[:],
                        start=False,
                        stop=(ci == CI - 1),
                        skip_group_check=True,
                    )
                g = small.tile([P, CI], F32, tag="g", name=f"g{b}")
                nc.scalar.activation(out=g[:], in_=p2[:], func=AF.Sigmoid)
                # single-producer copy: the muls wait on DVE only
                g2 = small.tile([P, CI], F32, tag="g2", name=f"g2_{b}")
                nc.vector.tensor_copy(out=g2[:], in_=g[:])

                for ci in range(CI):
                    yt = yp.tile([P, S], F32, tag="yt", name=f"yt{b}_{ci}")
                    nc.vector.tensor_scalar_mul(
                        out=yt[:],
                        in0=xt[:, ci, :],
                        scalar1=g2[:, ci : ci + 1],
                    )
                    out_eng = (
                        nc.scalar if (b == BPC - 1 and ci % 2 == 1) else nc.gpsimd
                    )
                    out_eng.dma_start(out=y[b, ci], in_=yt[:])
    nc.compile()
    return nc


_RUNNER = None


def _make_runner(nc):
    """jit(shard_map) over the bass_exec custom call — the same lowering
    run_bass_kernel_spmd uses under axon, but built once and cached so
    repeated kernel() calls reuse one loaded executable (loading a second
    copy of the NEFF in the same process wedges the device)."""
    import jax
    from jax.sharding import Mesh, PartitionSpec
    from jax.experimental.shard_map import shard_map
    from concourse.bass2jax import (
        _bass_exec_p,
        install_neuronx_cc_hook,
        partition_id_tensor,
    )

    install_neuronx_cc_hook()
    partition_name = nc.partition_id_tensor.name if nc.partition_id_tensor else None
    in_names, out_names, out_avals = [], [], []
    for alloc in nc.m.functions[0].allocations:
        if not isinstance(alloc, mybir.MemoryLocationSet):
            continue
        name = alloc.memorylocations[0].name
        if alloc.kind == "ExternalInput":
            if name != partition_name:
                in_names.append(name)
        elif alloc.kind == "ExternalOutput":
            out_names.append(name)
            out_avals.append(
                jax.core.ShapedArray(
                    tuple(alloc.tensor_shape), mybir.dt.np(alloc.dtype)
                )
            )
    all_in = in_names + out_names
    if partition_name is not None:
        all_in.append(partition_name)

    def _body(*args):
        operands = list(args)
        if partition_name is not None:
            operands.append(partition_id_tensor())
        outs = _bass_exec_p.bind(
            *operands,
            out_avals=tuple(out_avals),
            in_names=tuple(all_in),
            out_names=tuple(out_names),
            lowering_input_output_aliases=(),
            sim_require_finite=True,
            sim_require_nnan=True,
            nc=nc,
        )
        return tuple(outs)

    devices = jax.devices()[:N_CORES]
    mesh = Mesh(np.asarray(devices), ("core",))
    n_args = len(in_names) + len(out_names)
    fn = jax.jit(
        shard_map(
            _body,
            mesh=mesh,
            in_specs=(PartitionSpec("core"),) * n_args,
            out_specs=(PartitionSpec("core"),) * len(out_names),
            check_rep=False,
        ),
        keep_unused=True,
    )
    assert in_names == ["x", "w1t", "w2t"] and out_names == ["y"], (
        in_names,
        out_names,
    )
    return fn


def kernel(x, w1, w2, **_ignored):
    global _NC_CACHE, _RUNNER
    x = np.ascontiguousarray(np.asarray(x, dtype=np.float32))
    w1 = np.asarray(w1, dtype=np.float32)  # [HID, C]
    w2 = np.asarray(w2, dtype=np.float32)  # [C, HID]

    # SBUF layouts, pretransposed on host
    w1t = np.ascontiguousarray(
        w1.T.reshape(CI, P, HID).transpose(1, 0, 2)
    )  # [P, CI, HID]; w1t[p, ci, h] = w1[h, ci*128+p]
    w2t = np.ascontiguousarray(w2.T)  # [HID, C]

    if _NC_CACHE is None:
        _NC_CACHE = _build_bass()
    if _RUNNER is None:
        _RUNNER = _make_runner(_NC_CACHE)

    # global inputs concatenated on axis 0; shard_map hands each core its slice
    xs = x.reshape(N_CORES * BPC, CI, P, S)
    w1ts = np.concatenate([w1t] * N_CORES, axis=0)
    w2ts = np.concatenate([w2t] * N_CORES, axis=0)
    ybuf = np.zeros_like(xs)
    (y,) = _RUNNER(xs, w1ts, w2ts, ybuf)
    return np.asarray(y).reshape(B, C, 56, 56)


# revision 30
# speedup vs baseline: 1.1430x; 1.0016x over previous
"""CAM (channel-attention) kernel for Trainium2, data-parallel over batch on 8 cores.

Reference computation (per sample b):
    avg[c] = mean over spatial of x[b, c, :, :]
    mx[c]  = max  over spatial of x[b, c, :, :]
    gate   = sigmoid(W2 @ relu(W1 @ avg) + W2 @ relu(W1 @ mx))
    y[b]   = x[b] * gate[:, None, None]

Design (per core, 4 samples; memory-roofline bound at ~360 GB/s/core):
  - Each sample is SBUF-resident as [128 partitions, 4 channel-groups, 3136]
    (channel c = ci*128 + p): x is read from HBM once and written back once,
    the minimum possible traffic (~51 MB/core -> ~140 us at roofline).
  - Input DMAs ride the SP HW-DGE ring, output DMAs the Pool SWDGE ring:
    a DMA occupies its issuing ring for the whole transfer (~4.8 us per
    1.6 MB), so input and output streams must live on different rings, and
    the ACT ring is kept free for the mean-reduction activations. At the
    edges (first sample's loads, last sample's stores) the otherwise-idle
    ACT ring takes half the transfers, halving pipeline fill and drain.
  - The per-channel mean comes free on ScalarE: activation(Copy, scale=1/S)
    with accum_out, while VectorE does reduce_max. The scratch `out` of that
    activation is a small rotating dummy tile.
  - The tiny shared MLP runs on TensorE with host-pretransposed weights.
    relu(W1@mean) = relu(W1@sum)/S (positive homogeneity) and W2 is linear,
    so both branches merge into one [32,1] vector before layer 2:
        layer1: psum[32,2] += w1t[:,ci,:].T @ [mean | max], ci=0..3
        hsum = relu(psum)[:,0] + relu(psum)[:,1]
        layer2 per ci: p2[:,ci] = w2t[:,ci*128:].T @ hsum ; gate = sigmoid(p2)
  - Gating multiply on VectorE tensor_scalar (2x fp32 SBUF mode) into fresh
    output tiles, then streamed out per channel-group.
  - A zeroing matmul opens each PSUM accumulation group and warm-up matmuls
    touch the weight tiles once: every hot instruction then needs at most
    one semaphore wait, minimizing Bacc's EventSemaphore splitting (this
    toolchain allows exactly ONE wait slot per hardware instruction).
  - Built on Bacc: nc.compile() legalizes any remaining multi-wait
    instructions into EventSemaphore prefixes.
"""

import numpy as np

import concourse.bacc as bacc
import concourse.bass as bass
import concourse.tile as tile
from concourse import mybir

N_CORES = 8
B = 32
C = 512
S = 56 * 56  # 3136
BPC = B // N_CORES  # samples per core
P = 128
CI = C // P  # channel groups of 128
HID = 32

F32 = mybir.dt.float32
AF = mybir.ActivationFunctionType

LAST_RESULTS = None  # BassKernelResults of the most recent run (for test harness)
_NC_CACHE = None


def _build_bass():
    nc = bacc.Bacc()
    x = nc.dram_tensor("x", (BPC, CI, P, S), F32, kind="ExternalInput")
    w1t = nc.dram_tensor("w1t", (P, CI, HID), F32, kind="ExternalInput")
    w2t = nc.dram_tensor("w2t", (HID, C), F32, kind="ExternalInput")
    y = nc.dram_tensor("y", (BPC, CI, P, S), F32, kind="ExternalOutput")

    with tile.TileContext(nc) as tc:
        with (
            tc.tile_pool(name="xp", bufs=2) as xp,
            tc.tile_pool(name="yp", bufs=4) as yp,
            tc.tile_pool(name="consts", bufs=1) as consts,
            tc.tile_pool(name="small", bufs=4) as small,
            tc.tile_pool(name="dump", bufs=2) as dump,
            tc.tile_pool(name="ps1", bufs=4, space=bass.MemorySpace.PSUM) as ps1,
            tc.tile_pool(name="ps2", bufs=4, space=bass.MemorySpace.PSUM) as ps2,
        ):
            w1t_sb = consts.tile([P, CI, HID], F32)
            nc.sync.dma_start(out=w1t_sb[:], in_=w1t[:])
            w2t_sb = consts.tile([HID, C], F32)
            nc.sync.dma_start(out=w2t_sb[:], in_=w2t[:])
            zeros = consts.tile([P, CI], F32)
            nc.vector.memset(zeros[:], 0.0)

            # PE observes the two weight-DMA semaphores here, once.
            pw = ps1.tile([HID, 2], F32, tag="p1", name="pw")
            nc.tensor.matmul(pw[:, 0:1], w1t_sb[:, 0, :], w1t_sb[:, 0, 0:1])
            pw2 = ps2.tile([P, CI], F32, tag="p2", name="pw2")
            nc.tensor.matmul(pw2[:, 0:1], w2t_sb[:, 0:P], w2t_sb[:, 0:1])

            for b in range(BPC):
                xt = xp.tile([P, CI, S], F32, tag="xt", name=f"xt{b}")
                stats = small.tile([P, CI, 2], F32, tag="stats", name=f"st{b}")
                for ci in range(CI):
                    in_eng = nc.scalar if (b == 0 and ci == 1) else nc.sync
                    in_eng.dma_start(out=xt[:, ci, :], in_=x[b, ci])
                for ci in range(CI):
                    # ScalarE: accum_out = sum(x/S) = mean; dmy is scratch
                    dmy = dump.tile([P, S], F32, tag="dmy", name=f"dmy{b}_{ci}")
                    nc.scalar.activation(
                        out=dmy[:],
                        in_=xt[:, ci, :],
                        func=AF.Copy,
                        scale=1.0 / S,
                        accum_out=stats[:, ci, 0:1],
                    )
                    nc.vector.reduce_max(
                        out=stats[:, ci, 1:2],
                        in_=xt[:, ci, :],
                        axis=mybir.AxisListType.X,
                    )

                # layer 1: psum [32, 2] = sum_ci W1[:, ci-block] @ [mean | max]
                p1 = ps1.tile([HID, 2], F32, tag="p1", name=f"p1_{b}")
                nc.tensor.matmul(
                    p1[:], w1t_sb[:, 0, :], zeros[:, 0:2], start=True, stop=False
                )
                for ci in range(CI):
                    nc.tensor.matmul(
                        p1[:],
                        w1t_sb[:, ci, :],
                        stats[:, ci, :],
                        start=False,
                        stop=(ci == CI - 1),
                    )
                h = small.tile([HID, 2], F32, tag="h", name=f"h{b}")
                nc.scalar.activation(out=h[:], in_=p1[:], func=AF.Relu)
                # W2 is linear: merge branches before layer 2
                hsum = small.tile([HID, 1], F32, tag="hsum", name=f"hs{b}")
                nc.vector.tensor_scalar(
                    out=hsum[:],
                    in0=h[:, 0:1],
                    scalar1=1.0,
                    scalar2=h[:, 1:2],
                    op0=mybir.AluOpType.mult,
                    op1=mybir.AluOpType.add,
                )

                # layer 2: four matmuls into disjoint columns of one PSUM bank
                p2 = ps2.tile([P, CI], F32, tag="p2", name=f"p2_{b}")
                nc.tensor.matmul(
                    p2[:],
                    w2t_sb[:, 0:P],
                    zeros[:HID, 0:CI],
                    start=True,
                    stop=False,
                    skip_group_check=True,
                )
                for ci in range(CI):
                    nc.tensor.matmul(
                        p2[:, ci : ci + 1],
                        w2t_sb[:, ci * P : (ci + 1) * P],
                        hsum[:],
                        start=False,
                        stop=(ci == CI - 1),
                        skip_group_check=True,
                    )
                g = small.tile([P, CI], F32, tag="g", name=f"g{b}")
                nc.scalar.activation(out=g[:], in_=p2[:], func=AF.Sigmoid)
                # single-producer copy: the muls wait on DVE only
                g2 = small.tile([P, CI], F32, tag="g2", name=f"g2_{b}")
                nc.vector.tensor_copy(out=g2[:], in_=g[:])

                for ci in range(CI):
                    yt = yp.tile([P, S], F32, tag="yt", name=f"yt{b}_{ci}")
                    if ci == 0:
                        # one gating multiply per sample on ScalarE rebalances
                        # the DVE chain (the longest engine chain otherwise)
                        nc.scalar.activation(
                            out=yt[:],
                            in_=xt[:, ci, :],
                            func=AF.Copy,
                            scale=g[:, ci : ci + 1],
                        )
                    else:
                        nc.vector.tensor_scalar_mul(
                            out=yt[:],
                            in0=xt[:, ci, :],
                            scalar1=g2[:, ci : ci + 1],
                        )
                    out_eng = (
                        nc.scalar if (b == BPC - 1 and ci == 1) else nc.gpsimd
                    )
                    out_eng.dma_start(out=y[b, ci], in_=yt[:])
    nc.compile()
    return nc


_RUNNER = None


def _make_runner(nc):
    """jit(shard_map) over the bass_exec custom call — the same lowering
    run_bass_kernel_spmd uses under axon, but built once and cached so
    repeated kernel() calls reuse one loaded executable (loading a second
    copy of the NEFF in the same process wedges the device)."""
    import jax
    from jax.sharding import Mesh, PartitionSpec
    from jax.experimental.shard_map import shard_map
    from concourse.bass2jax import (
        _bass_exec_p,
        install_neuronx_cc_hook,
        partition_id_tensor,
    )

    install_neuronx_cc_hook()
    partition_name = nc.partition_id_tensor.name if nc.partition_id_tensor else None
    in_names, out_names, out_avals = [], [], []
    for alloc in nc.m.functions[0].allocations:
        if not isinstance(alloc, mybir.MemoryLocationSet):
            continue
        name = alloc.memorylocations[0].name
        if alloc.kind == "ExternalInput":
            if name != partition_name:
                in_names.append(name)
        elif alloc.kind == "ExternalOutput":
            out_names.append(name)
            out_avals.append(
                jax.core.ShapedArray(
                    tuple(alloc.tensor_shape), mybir.dt.np(alloc.dtype)
                )
            )
    all_in = in_names + out_names
    if partition_name is not None:
        all_in.append(partition_name)

    def _body(*args):
        operands = list(args)
        if partition_name is not None:
            operands.append(partition_id_tensor())
        outs = _bass_exec_p.bind(
            *operands,
            out_avals=tuple(out_avals),
            in_names=tuple(all_in),
            out_names=tuple(out_names),
            lowering_input_output_aliases=(),
            sim_require_finite=True,
            sim_require_nnan=True,
            nc=nc,
        )
        return tuple(outs)

    devices = jax.devices()[:N_CORES]
    mesh = Mesh(np.asarray(devices), ("core",))
    n_args = len(in_names) + len(out_names)
    fn = jax.jit(
        shard_map(
            _body,
            mesh=mesh,
            in_specs=(PartitionSpec("core"),) * n_args,
            out_specs=(PartitionSpec("core"),) * len(out_names),
            check_rep=False,
        ),
        keep_unused=True,
    )
    assert in_names == ["x", "w1t", "w2t"] and out_names == ["y"], (
        in_names,
        out_names,
    )
    return fn


def kernel(x, w1, w2, **_ignored):
    global _NC_CACHE, _RUNNER
    x = np.ascontiguousarray(np.asarray(x, dtype=np.float32))
    w1 = np.asarray(w1, dtype=np.float32)  # [HID, C]
    w2 = np.asarray(w2, dtype=np.float32)  # [C, HID]

    # SBUF layouts, pretransposed on host
    w1t = np.ascontiguousarray(
        w1.T.reshape(CI, P, HID).transpose(1, 0, 2)
    )  # [P, CI, HID]; w1t[p, ci, h] = w1[h, ci*128+p]
    w2t = np.ascontiguousarray(w2.T)  # [HID, C]

    if _NC_CACHE is None:
        _NC_CACHE = _build_bass()
    if _RUNNER is None:
        _RUNNER = _make_runner(_NC_CACHE)

    # global inputs concatenated on axis 0; shard_map hands each core its slice
    xs = x.reshape(N_CORES * BPC, CI, P, S)
    w1ts = np.concatenate([w1t] * N_CORES, axis=0)
    w2ts = np.concatenate([w2t] * N_CORES, axis=0)
    ybuf = np.zeros_like(xs)
    (y,) = _RUNNER(xs, w1ts, w2ts, ybuf)
    return np.asarray(y).reshape(B, C, 56, 56)


# revision 31
# speedup vs baseline: 1.1991x; 1.0491x over previous
"""CAM (channel-attention) kernel for Trainium2, data-parallel over batch on 8 cores.

Reference computation (per sample b):
    avg[c] = mean over spatial of x[b, c, :, :]
    mx[c]  = max  over spatial of x[b, c, :, :]
    gate   = sigmoid(W2 @ relu(W1 @ avg) + W2 @ relu(W1 @ mx))
    y[b]   = x[b] * gate[:, None, None]

Design (per core, 4 samples; memory-roofline bound at ~360 GB/s/core):
  - Each sample is SBUF-resident as [128 partitions, 4 channel-groups, 3136]
    (channel c = ci*128 + p): x is read from HBM once and written back once,
    the minimum possible traffic (~51 MB/core -> ~140 us at roofline).
  - Input DMAs ride the SP HW-DGE ring, output DMAs the Pool SWDGE ring:
    a DMA occupies its issuing ring for the whole transfer (~4.8 us per
    1.6 MB), so input and output streams must live on different rings, and
    the ACT ring is kept free for the mean-reduction activations. At the
    edges (first sample's loads, last sample's stores) the otherwise-idle
    ACT ring takes half the transfers, halving pipeline fill and drain.
  - The per-channel mean comes free on ScalarE: activation(Copy, scale=1/S)
    with accum_out, while VectorE does reduce_max. The scratch `out` of that
    activation is a small rotating dummy tile.
  - The tiny shared MLP runs on TensorE with host-pretransposed weights.
    relu(W1@mean) = relu(W1@sum)/S (positive homogeneity) and W2 is linear,
    so both branches merge into one [32,1] vector before layer 2:
        layer1: psum[32,2] += w1t[:,ci,:].T @ [mean | max], ci=0..3
        hsum = relu(psum)[:,0] + relu(psum)[:,1]
        layer2 per ci: p2[:,ci] = w2t[:,ci*128:].T @ hsum ; gate = sigmoid(p2)
  - Gating multiply on VectorE tensor_scalar (2x fp32 SBUF mode) into fresh
    output tiles, then streamed out per channel-group.
  - A zeroing matmul opens each PSUM accumulation group and warm-up matmuls
    touch the weight tiles once: every hot instruction then needs at most
    one semaphore wait, minimizing Bacc's EventSemaphore splitting (this
    toolchain allows exactly ONE wait slot per hardware instruction).
  - Built on Bacc: nc.compile() legalizes any remaining multi-wait
    instructions into EventSemaphore prefixes.
"""

import numpy as np

import concourse.bacc as bacc
import concourse.bass as bass
import concourse.tile as tile
from concourse import mybir

N_CORES = 8
B = 32
C = 512
S = 56 * 56  # 3136
BPC = B // N_CORES  # samples per core
P = 128
CI = C // P  # channel groups of 128
HID = 32

F32 = mybir.dt.float32
AF = mybir.ActivationFunctionType

LAST_RESULTS = None  # BassKernelResults of the most recent run (for test harness)
_NC_CACHE = None


def _build_bass():
    nc = bacc.Bacc()
    x = nc.dram_tensor("x", (BPC, CI, P, S), F32, kind="ExternalInput")
    w1t = nc.dram_tensor("w1t", (P, CI, HID), F32, kind="ExternalInput")
    w2t = nc.dram_tensor("w2t", (HID, C), F32, kind="ExternalInput")
    y = nc.dram_tensor("y", (BPC, CI, P, S), F32, kind="ExternalOutput")

    with tile.TileContext(nc) as tc:
        with (
            tc.tile_pool(name="xp", bufs=2) as xp,
            tc.tile_pool(name="yp", bufs=4) as yp,
            tc.tile_pool(name="consts", bufs=1) as consts,
            tc.tile_pool(name="small", bufs=4) as small,
            tc.tile_pool(name="dump", bufs=2) as dump,
            tc.tile_pool(name="ps1", bufs=4, space=bass.MemorySpace.PSUM) as ps1,
            tc.tile_pool(name="ps2", bufs=4, space=bass.MemorySpace.PSUM) as ps2,
        ):
            w1t_sb = consts.tile([P, CI, HID], F32)
            nc.sync.dma_start(out=w1t_sb[:], in_=w1t[:])
            w2t_sb = consts.tile([HID, C], F32)
            nc.sync.dma_start(out=w2t_sb[:], in_=w2t[:])
            zeros = consts.tile([P, CI], F32)
            nc.vector.memset(zeros[:], 0.0)

            # PE observes the two weight-DMA semaphores here, once.
            pw = ps1.tile([HID, 2], F32, tag="p1", name="pw")
            nc.tensor.matmul(pw[:, 0:1], w1t_sb[:, 0, :], w1t_sb[:, 0, 0:1])
            pw2 = ps2.tile([P, CI], F32, tag="p2", name="pw2")
            nc.tensor.matmul(pw2[:, 0:1], w2t_sb[:, 0:P], w2t_sb[:, 0:1])

            for b in range(BPC):
                xt = xp.tile([P, CI, S], F32, tag="xt", name=f"xt{b}")
                stats = small.tile([P, CI, 2], F32, tag="stats", name=f"st{b}")
                for ci in range(CI):
                    in_eng = nc.scalar if (b == 0 and ci == 1) else nc.sync
                    in_eng.dma_start(out=xt[:, ci, :], in_=x[b, ci])
                for ci in range(CI):
                    # ScalarE: accum_out = sum(x/S) = mean; dmy is scratch
                    dmy = dump.tile([P, S], F32, tag="dmy", name=f"dmy{b}_{ci}")
                    nc.scalar.activation(
                        out=dmy[:],
                        in_=xt[:, ci, :],
                        func=AF.Copy,
                        scale=1.0 / S,
                        accum_out=stats[:, ci, 0:1],
                    )
                    nc.vector.reduce_max(
                        out=stats[:, ci, 1:2],
                        in_=xt[:, ci, :],
                        axis=mybir.AxisListType.X,
                    )

                # layer 1: psum [32, 2] = sum_ci W1[:, ci-block] @ [mean | max]
                p1 = ps1.tile([HID, 2], F32, tag="p1", name=f"p1_{b}")
                nc.tensor.matmul(
                    p1[:], w1t_sb[:, 0, :], zeros[:, 0:2], start=True, stop=False
                )
                for ci in range(CI):
                    nc.tensor.matmul(
                        p1[:],
                        w1t_sb[:, ci, :],
                        stats[:, ci, :],
                        start=False,
                        stop=(ci == CI - 1),
                    )
                h = small.tile([HID, 2], F32, tag="h", name=f"h{b}")
                nc.scalar.activation(out=h[:], in_=p1[:], func=AF.Relu)
                # W2 is linear: merge branches before layer 2
                hsum = small.tile([HID, 1], F32, tag="hsum", name=f"hs{b}")
                nc.vector.tensor_scalar(
                    out=hsum[:],
                    in0=h[:, 0:1],
                    scalar1=1.0,
                    scalar2=h[:, 1:2],
                    op0=mybir.AluOpType.mult,
                    op1=mybir.AluOpType.add,
                )

                # layer 2: four matmuls into disjoint columns of one PSUM bank
                p2 = ps2.tile([P, CI], F32, tag="p2", name=f"p2_{b}")
                nc.tensor.matmul(
                    p2[:],
                    w2t_sb[:, 0:P],
                    zeros[:HID, 0:CI],
                    start=True,
                    stop=False,
                    skip_group_check=True,
                )
                for ci in range(CI):
                    nc.tensor.matmul(
                        p2[:, ci : ci + 1],
                        w2t_sb[:, ci * P : (ci + 1) * P],
                        hsum[:],
                        start=False,
                        stop=(ci == CI - 1),
                        skip_group_check=True,
                    )
                g = small.tile([P, CI], F32, tag="g", name=f"g{b}")
                nc.scalar.activation(out=g[:], in_=p2[:], func=AF.Sigmoid)
                # single-producer copy: the muls wait on DVE only
                g2 = small.tile([P, CI], F32, tag="g2", name=f"g2_{b}")
                nc.vector.tensor_copy(out=g2[:], in_=g[:])

                for ci in range(CI):
                    yt = yp.tile([P, S], F32, tag="yt", name=f"yt{b}_{ci}")
                    if ci == 0:
                        # one gating multiply per sample on ScalarE rebalances
                        # the DVE chain (the longest engine chain otherwise)
                        nc.scalar.activation(
                            out=yt[:],
                            in_=xt[:, ci, :],
                            func=AF.Copy,
                            scale=g[:, ci : ci + 1],
                        )
                    else:
                        nc.vector.tensor_scalar_mul(
                            out=yt[:],
                            in0=xt[:, ci, :],
                            scalar1=g2[:, ci : ci + 1],
                        )
                    out_eng = (
                        nc.scalar if (b == BPC - 1 and ci % 2 == 1) else nc.gpsimd
                    )
                    out_eng.dma_start(out=y[b, ci], in_=yt[:])
    nc.compile()
    return nc


_RUNNER = None


def _make_runner(nc):
    """jit(shard_map) over the bass_exec custom call — the same lowering
    run_bass_kernel_spmd uses under axon, but built once and cached so
    repeated kernel() calls reuse one loaded executable (loading a second
    copy of the NEFF in the same process wedges the device)."""
    import jax
    from jax.sharding import Mesh, PartitionSpec
    from jax.experimental.shard_map import shard_map
    from concourse.bass2jax import (
        _bass_exec_p,
        install_neuronx_cc_hook,
        partition_id_tensor,
    )

    install_neuronx_cc_hook()
    partition_name = nc.partition_id_tensor.name if nc.partition_id_tensor else None
    in_names, out_names, out_avals = [], [], []
    for alloc in nc.m.functions[0].allocations:
        if not isinstance(alloc, mybir.MemoryLocationSet):
            continue
        name = alloc.memorylocations[0].name
        if alloc.kind == "ExternalInput":
            if name != partition_name:
                in_names.append(name)
        elif alloc.kind == "ExternalOutput":
            out_names.append(name)
            out_avals.append(
                jax.core.ShapedArray(
                    tuple(alloc.tensor_shape), mybir.dt.np(alloc.dtype)
                )
            )
    all_in = in_names + out_names
    if partition_name is not None:
        all_in.append(partition_name)

    def _body(*args):
        operands = list(args)
        if partition_name is not None:
            operands.append(partition_id_tensor())
        outs = _bass_exec_p.bind(
            *operands,
            out_avals=tuple(out_avals),
            in_names=tuple(all_in),
            out_names=tuple(out_names),
            lowering_input_output_aliases=(),
            sim_require_finite=True,
            sim_require_nnan=True,
            nc=nc,
        )
        return tuple(outs)

    devices = jax.devices()[:N_CORES]
    mesh = Mesh(np.asarray(devices), ("core",))
    n_args = len(in_names) + len(out_names)
    fn = jax.jit(
        shard_map(
            _body,
            mesh=mesh,
            in_specs=(PartitionSpec("core"),) * n_args,
            out_specs=(PartitionSpec("core"),) * len(out_names),
            check_rep=False,
        ),
        keep_unused=True,
    )
    assert in_names == ["x", "w1t", "w2t"] and out_names == ["y"], (
        in_names,
        out_names,
    )
    return fn


def kernel(x, w1, w2, **_ignored):
    global _NC_CACHE, _RUNNER
    x = np.ascontiguousarray(np.asarray(x, dtype=np.float32))
    w1 = np.asarray(w1, dtype=np.float32)  # [HID, C]
    w2 = np.asarray(w2, dtype=np.float32)  # [C, HID]

    # SBUF layouts, pretransposed on host
    w1t = np.ascontiguousarray(
        w1.T.reshape(CI, P, HID).transpose(1, 0, 2)
    )  # [P, CI, HID]; w1t[p, ci, h] = w1[h, ci*128+p]
    w2t = np.ascontiguousarray(w2.T)  # [HID, C]

    if _NC_CACHE is None:
        _NC_CACHE = _build_bass()
    if _RUNNER is None:
        _RUNNER = _make_runner(_NC_CACHE)

    # global inputs concatenated on axis 0; shard_map hands each core its slice
    xs = x.reshape(N_CORES * BPC, CI, P, S)
    w1ts = np.concatenate([w1t] * N_CORES, axis=0)
    w2ts = np.concatenate([w2t] * N_CORES, axis=0)
    ybuf = np.zeros_like(xs)
    (y,) = _RUNNER(xs, w1ts, w2ts, ybuf)
    return np.asarray(y).reshape(B, C, 56, 56)


# revision 32
# speedup vs baseline: 1.2312x; 1.0267x over previous
"""CAM (channel-attention) kernel for Trainium2, data-parallel over batch on 8 cores.

Reference computation (per sample b):
    avg[c] = mean over spatial of x[b, c, :, :]
    mx[c]  = max  over spatial of x[b, c, :, :]
    gate   = sigmoid(W2 @ relu(W1 @ avg) + W2 @ relu(W1 @ mx))
    y[b]   = x[b] * gate[:, None, None]

Design (per core, 4 samples; memory-roofline bound at ~360 GB/s/core):
  - Each sample is SBUF-resident as [128 partitions, 4 channel-groups, 3136]
    (channel c = ci*128 + p): x is read from HBM once and written back once,
    the minimum possible traffic (~51 MB/core -> ~140 us at roofline).
  - Input DMAs ride the SP HW-DGE ring, output DMAs the Pool SWDGE ring:
    a DMA occupies its issuing ring for the whole transfer (~4.8 us per
    1.6 MB), so input and output streams must live on different rings, and
    the ACT ring is kept free for the mean-reduction activations. At the
    edges (first sample's loads, last sample's stores) the otherwise-idle
    ACT ring takes half the transfers, halving pipeline fill and drain.
  - The per-channel mean comes free on ScalarE: activation(Copy, scale=1/S)
    with accum_out, while VectorE does reduce_max. The scratch `out` of that
    activation is a small rotating dummy tile.
  - The tiny shared MLP runs on TensorE with host-pretransposed weights.
    relu(W1@mean) = relu(W1@sum)/S (positive homogeneity) and W2 is linear,
    so both branches merge into one [32,1] vector before layer 2:
        layer1: psum[32,2] += w1t[:,ci,:].T @ [mean | max], ci=0..3
        hsum = relu(psum)[:,0] + relu(psum)[:,1]
        layer2 per ci: p2[:,ci] = w2t[:,ci*128:].T @ hsum ; gate = sigmoid(p2)
  - Gating multiply on VectorE tensor_scalar (2x fp32 SBUF mode) into fresh
    output tiles, then streamed out per channel-group.
  - A zeroing matmul opens each PSUM accumulation group and warm-up matmuls
    touch the weight tiles once: every hot instruction then needs at most
    one semaphore wait, minimizing Bacc's EventSemaphore splitting (this
    toolchain allows exactly ONE wait slot per hardware instruction).
  - Built on Bacc: nc.compile() legalizes any remaining multi-wait
    instructions into EventSemaphore prefixes.
"""

import numpy as np

import concourse.bacc as bacc
import concourse.bass as bass
import concourse.tile as tile
from concourse import mybir

N_CORES = 8
B = 32
C = 512
S = 56 * 56  # 3136
BPC = B // N_CORES  # samples per core
P = 128
CI = C // P  # channel groups of 128
HID = 32

F32 = mybir.dt.float32
AF = mybir.ActivationFunctionType

LAST_RESULTS = None  # BassKernelResults of the most recent run (for test harness)
_NC_CACHE = None


def _build_bass():
    nc = bacc.Bacc()
    x = nc.dram_tensor("x", (BPC, CI, P, S), F32, kind="ExternalInput")
    w1t = nc.dram_tensor("w1t", (P, CI, HID), F32, kind="ExternalInput")
    w2t = nc.dram_tensor("w2t", (HID, C), F32, kind="ExternalInput")
    y = nc.dram_tensor("y", (BPC, CI, P, S), F32, kind="ExternalOutput")

    with tile.TileContext(nc) as tc:
        with (
            tc.tile_pool(name="xp", bufs=2) as xp,
            tc.tile_pool(name="yp", bufs=4) as yp,
            tc.tile_pool(name="consts", bufs=1) as consts,
            tc.tile_pool(name="small", bufs=4) as small,
            tc.tile_pool(name="dump", bufs=2) as dump,
            tc.tile_pool(name="ps1", bufs=4, space=bass.MemorySpace.PSUM) as ps1,
            tc.tile_pool(name="ps2", bufs=4, space=bass.MemorySpace.PSUM) as ps2,
        ):
            w1t_sb = consts.tile([P, CI, HID], F32)
            nc.sync.dma_start(out=w1t_sb[:], in_=w1t[:])
            w2t_sb = consts.tile([HID, C], F32)
            nc.sync.dma_start(out=w2t_sb[:], in_=w2t[:])
            zeros = consts.tile([P, CI], F32)
            nc.vector.memset(zeros[:], 0.0)

            # PE observes the two weight-DMA semaphores here, once.
            pw = ps1.tile([HID, 2], F32, tag="p1", name="pw")
            nc.tensor.matmul(pw[:, 0:1], w1t_sb[:, 0, :], w1t_sb[:, 0, 0:1])
            pw2 = ps2.tile([P, CI], F32, tag="p2", name="pw2")
            nc.tensor.matmul(pw2[:, 0:1], w2t_sb[:, 0:P], w2t_sb[:, 0:1])

            for b in range(BPC):
                xt = xp.tile([P, CI, S], F32, tag="xt", name=f"xt{b}")
                stats = small.tile([P, CI, 2], F32, tag="stats", name=f"st{b}")
                for ci in range(CI):
                    in_eng = nc.scalar if (b == 0 and ci == 1) else nc.sync
                    in_eng.dma_start(out=xt[:, ci, :], in_=x[b, ci])
                for ci in range(CI):
                    # ScalarE: accum_out = sum(x/S) = mean; dmy is scratch
                    dmy = dump.tile([P, S], F32, tag="dmy", name=f"dmy{b}_{ci}")
                    nc.scalar.activation(
                        out=dmy[:],
                        in_=xt[:, ci, :],
                        func=AF.Copy,
                        scale=1.0 / S,
                        accum_out=stats[:, ci, 0:1],
                    )
                    nc.vector.reduce_max(
                        out=stats[:, ci, 1:2],
                        in_=xt[:, ci, :],
                        axis=mybir.AxisListType.X,
                    )

                # layer 1: psum [32, 2] = sum_ci W1[:, ci-block] @ [mean | max]
                p1 = ps1.tile([HID, 2], F32, tag="p1", name=f"p1_{b}")
                nc.tensor.matmul(
                    p1[:], w1t_sb[:, 0, :], zeros[:, 0:2], start=True, stop=False
                )
                for ci in range(CI):
                    nc.tensor.matmul(
                        p1[:],
                        w1t_sb[:, ci, :],
                        stats[:, ci, :],
                        start=False,
                        stop=(ci == CI - 1),
                    )
                h = small.tile([HID, 2], F32, tag="h", name=f"h{b}")
                nc.scalar.activation(out=h[:], in_=p1[:], func=AF.Relu)
                # W2 is linear: merge branches before layer 2
                hsum = small.tile([HID, 1], F32, tag="hsum", name=f"hs{b}")
                nc.vector.tensor_scalar(
                    out=hsum[:],
                    in0=h[:, 0:1],
                    scalar1=1.0,
                    scalar2=h[:, 1:2],
                    op0=mybir.AluOpType.mult,
                    op1=mybir.AluOpType.add,
                )

                # layer 2: four matmuls into disjoint columns of one PSUM bank
                p2 = ps2.tile([P, CI], F32, tag="p2", name=f"p2_{b}")
                nc.tensor.matmul(
                    p2[:],
                    w2t_sb[:, 0:P],
                    zeros[:HID, 0:CI],
                    start=True,
                    stop=False,
                    skip_group_check=True,
                )
                for ci in range(CI):
                    nc.tensor.matmul(
                        p2[:, ci : ci + 1],
                        w2t_sb[:, ci * P : (ci + 1) * P],
                        hsum[:],
                        start=False,
                        stop=(ci == CI - 1),
                        skip_group_check=True,
                    )
                g = small.tile([P, CI], F32, tag="g", name=f"g{b}")
                nc.scalar.activation(out=g[:], in_=p2[:], func=AF.Sigmoid)
                # single-producer copy: the muls wait on DVE only
                g2 = small.tile([P, CI], F32, tag="g2", name=f"g2_{b}")
                nc.vector.tensor_copy(out=g2[:], in_=g[:])

                for ci in range(CI):
                    yt = yp.tile([P, S], F32, tag="yt", name=f"yt{b}_{ci}")
                    if ci == 0:
                        # one gating multiply per sample on ScalarE rebalances
                        # the DVE chain (the longest engine chain otherwise)
                        nc.scalar.activation(
                            out=yt[:],
                            in_=xt[:, ci, :],
                            func=AF.Copy,
                            scale=g[:, ci : ci + 1],
                        )
                    else:
                        nc.vector.tensor_scalar_mul(
                            out=yt[:],
                            in0=xt[:, ci, :],
                            scalar1=g2[:, ci : ci + 1],
                        )
                    # Tail drain on three rings: SP's ring is done loading by
                    # the time the last samples store, so it takes late
                    # stores; ACT's ring (idle at the end) takes two more.
                    if b == BPC - 1:
                        out_eng = (nc.gpsimd, nc.scalar, nc.sync, nc.scalar)[ci]
                    elif b == BPC - 2 and ci == CI - 1:
                        out_eng = nc.sync
                    else:
                        out_eng = nc.gpsimd
                    out_eng.dma_start(out=y[b, ci], in_=yt[:])
    nc.compile()
    return nc


_RUNNER = None


def _make_runner(nc):
    """jit(shard_map) over the bass_exec custom call — the same lowering
    run_bass_kernel_spmd uses under axon, but built once and cached so
    repeated kernel() calls reuse one loaded executable (loading a second
    copy of the NEFF in the same process wedges the device)."""
    import jax
    from jax.sharding import Mesh, PartitionSpec
    from jax.experimental.shard_map import shard_map
    from concourse.bass2jax import (
        _bass_exec_p,
        install_neuronx_cc_hook,
        partition_id_tensor,
    )

    install_neuronx_cc_hook()
    partition_name = nc.partition_id_tensor.name if nc.partition_id_tensor else None
    in_names, out_names, out_avals = [], [], []
    for alloc in nc.m.functions[0].allocations:
        if not isinstance(alloc, mybir.MemoryLocationSet):
            continue
        name = alloc.memorylocations[0].name
        if alloc.kind == "ExternalInput":
            if name != partition_name:
                in_names.append(name)
        elif alloc.kind == "ExternalOutput":
            out_names.append(name)
            out_avals.append(
                jax.core.ShapedArray(
                    tuple(alloc.tensor_shape), mybir.dt.np(alloc.dtype)
                )
            )
    all_in = in_names + out_names
    if partition_name is not None:
        all_in.append(partition_name)

    def _body(*args):
        operands = list(args)
        if partition_name is not None:
            operands.append(partition_id_tensor())
        outs = _bass_exec_p.bind(
            *operands,
            out_avals=tuple(out_avals),
            in_names=tuple(all_in),
            out_names=tuple(out_names),
            lowering_input_output_aliases=(),
            sim_require_finite=True,
            sim_require_nnan=True,
            nc=nc,
        )
        return tuple(outs)

    devices = jax.devices()[:N_CORES]
    mesh = Mesh(np.asarray(devices), ("core",))
    n_args = len(in_names) + len(out_names)
    fn = jax.jit(
        shard_map(
            _body,
            mesh=mesh,
            in_specs=(PartitionSpec("core"),) * n_args,
            out_specs=(PartitionSpec("core"),) * len(out_names),
            check_rep=False,
        ),
        keep_unused=True,
    )
    assert in_names == ["x", "w1t", "w2t"] and out_names == ["y"], (
        in_names,
        out_names,
    )
    return fn


def kernel(x, w1, w2, **_ignored):
    global _NC_CACHE, _RUNNER
    x = np.ascontiguousarray(np.asarray(x, dtype=np.float32))
    w1 = np.asarray(w1, dtype=np.float32)  # [HID, C]
    w2 = np.asarray(w2, dtype=np.float32)  # [C, HID]

    # SBUF layouts, pretransposed on host
    w1t = np.ascontiguousarray(
        w1.T.reshape(CI, P, HID).transpose(1, 0, 2)
    )  # [P, CI, HID]; w1t[p, ci, h] = w1[h, ci*128+p]
    w2t = np.ascontiguousarray(w2.T)  # [HID, C]

    if _NC_CACHE is None:
        _NC_CACHE = _build_bass()
    if _RUNNER is None:
        _RUNNER = _make_runner(_NC_CACHE)

    # global inputs concatenated on axis 0; shard_map hands each core its slice
    xs = x.reshape(N_CORES * BPC, CI, P, S)
    w1ts = np.concatenate([w1t] * N_CORES, axis=0)
    w2ts = np.concatenate([w2t] * N_CORES, axis=0)
    ybuf = np.zeros_like(xs)
    (y,) = _RUNNER(xs, w1ts, w2ts, ybuf)
    return np.asarray(y).reshape(B, C, 56, 56)
